# revision 1
# baseline (speedup 1.0000x reference)
"""Trainium2 Bass kernel for nn_MultiHeadAttention (B=4, S=2048, D=1024,
H=16, DK=DV=64) with key-padding + causal mask, exp-without-max softmax.

Sharding: 8 cores = (batch b = core//2) x (head half = core%2, 8 heads each).
Each core computes its batch's projections for its 8 heads and the full
attention for those heads; host reassembles [B, S, H*DV].

Device-side design (per core), all matmuls in fp32r (full-rate fp32):
 - host passes X^T = Q[b].T etc. [D, S] so the contraction dim (D) lands on
   partitions; W^T likewise.
 - qT/kT computed as W^T-chunk.T @ X^T-chunk -> [feat, S] (feat on
   partitions, 64 per head), v computed natural [S, feat].
 - mv1 = [v * mask | mask] per k-tile: folds the key-padding mask into both
   the PV numerator and the denominator (ones column -> masked row sums).
 - scores^T tile [128 k, <=512 q] = kT-tile.T @ qT-chunk; ACT exp with
   scale=1/8 fused; causal via 0/1 triangular masks on the diagonal tiles
   (host precomputed), with the dead low-q region trimmed to the 256-col
   fp32r minimum; off-diagonal upper tiles never computed.
 - ctx^T [65, 512] accumulated in PSUM over k-tiles; row 64 = masked
   exp-sums. Normalization: regather sums [1,512]->[4,128] via SBUF DMA,
   reciprocal, one PE transpose -> per-token scalars; ctx^T transposed back
   to token-major via PE transposes and scaled with tensor_scalar.
 - loops are software-pipelined per 512-token q-chunk group: project the
   group's tokens (q/k/v for all heads), then run attention for all 8 heads
   at that q-chunk, with the next group's projections emitted between head
   pairs. The Tile scheduler fills PE gaps during the (ACT-paced) attention
   with projection matmuls, keeping the tensor engine dense.
 - q^T is stored per head zero-padded to K=128 partitions: the Trainium2
   PE streams K<=64 matmuls at HALF rate (activity-based clock gate), so
   QK^T runs with K=128 where the 64 dead rows multiply zeros (exact) and
   the co-packed other head's kT rows are cancelled by those zeros.
"""

import sys

sys.path.insert(0, "/opt/trn_rl_repo")

import numpy as np

import concourse.bass as bass
import concourse.mybir as mybir
import concourse.tile as tile
from concourse import bacc
from concourse.bass_utils import run_bass_kernel_spmd

F32 = mybir.dt.float32
F32R = mybir.dt.float32r
EXP = mybir.ActivationFunctionType.Exp
COPY = mybir.ActivationFunctionType.Copy
IDENT_FN = mybir.ActivationFunctionType.Identity

B, S, D = 4, 2048, 1024
H, DK, DV = 16, 64, 64
HPC = 8  # heads per core
FPC = HPC * DK  # projected features per core (512)
NTT = S // 128  # 16 token tiles
NQC = S // 512  # 4 q-chunks
TC = 512  # projection token-chunk size
NDC = D // 128  # 8 contraction chunks
SCALE = 1.0 / np.sqrt(DK)
# dead low-q columns trimmed off diagonal tiles (fp32r needs N>=256)
DIAG_OFF = [0, 128, 256, 256]


def build_nc(has_bq=False, has_bk=False, has_bv=False):
    nc = bacc.Bacc()

    xt = {n: nc.dram_tensor(f"xt{n}", [D, S], F32R, kind="ExternalInput") for n in "qkv"}
    wt = {n: nc.dram_tensor(f"wt{n}", [D, FPC], F32R, kind="ExternalInput") for n in "qkv"}
    mask_d = nc.dram_tensor("maskf", [S], F32, kind="ExternalInput")
    tri_d = nc.dram_tensor("tri", [4, 128, 512], F32R, kind="ExternalInput")
    id_d = nc.dram_tensor("ident", [128, 128], F32R, kind="ExternalInput")
    bq_d = nc.dram_tensor("bq", [FPC], F32, kind="ExternalInput") if has_bq else None
    bk_d = nc.dram_tensor("bk", [FPC], F32, kind="ExternalInput") if has_bk else None
    bv_d = nc.dram_tensor("bv", [DV], F32, kind="ExternalInput") if has_bv else None
    out_d = nc.dram_tensor("out", [S, FPC], F32R, kind="ExternalOutput")

    with tile.TileContext(nc) as tc:
        with (
            tc.tile_pool(name="const", bufs=1) as cpool,
            tc.tile_pool(name="wtp", bufs=1) as wtpool,
            tc.tile_pool(name="xtp", bufs=3) as xtpool,
            tc.tile_pool(name="big", bufs=1) as big,
            tc.tile_pool(name="qt", bufs=2) as qtpool,
            tc.tile_pool(name="e", bufs=3) as epool,
            tc.tile_pool(name="fin", bufs=4) as fin,
            tc.tile_pool(name="mm", bufs=4, space="PSUM") as psmm,
            tc.tile_pool(name="ctx", bufs=2, space="PSUM") as psctx,
            tc.tile_pool(name="tp", bufs=2, space="PSUM") as pstp,
        ):
            # ---------------- constants
            tri = cpool.tile([128, 4, 512], F32R)
            nc.sync.dma_start(
                out=tri,
                in_=bass.AP(tensor=tri_d, offset=0, ap=[[512, 128], [512 * 128, 4], [1, 512]]),
            )
            zeros_sb = cpool.tile([64, 512], F32)
            nc.vector.memset(zeros_sb, 0.0)
            ident = cpool.tile([128, 128], F32R)
            nc.sync.dma_start(out=ident, in_=id_d[:, :])
            maskcol = cpool.tile([128, NTT], F32)
            nc.sync.dma_start(
                out=maskcol,
                in_=bass.AP(tensor=mask_d, offset=0, ap=[[1, 128], [128, NTT]]),
            )
            bias_sb = {}
            for n, b_d in (("q", bq_d), ("k", bk_d)):
                if b_d is not None:
                    t = cpool.tile([128, 4], F32)
                    nc.sync.dma_start(
                        out=t, in_=bass.AP(tensor=b_d, offset=0, ap=[[1, 128], [128, 4]])
                    )
                    bias_sb[n] = t
            if bv_d is not None:
                bv_b = cpool.tile([128, FPC], F32)
                nc.sync.dma_start(
                    out=bv_b,
                    in_=bass.AP(tensor=bv_d, offset=0, ap=[[0, 128], [0, HPC], [1, DV]]),
                )

            # PE warm-up: ~6us of dummy same-weight matmuls on the small
            # constant tiles (which arrive well before the bulk input DMAs).
            # They fill the initial DMA wait and hold the PE activity monitor
            # at full clock so the first real matmuls start warm. Results are
            # never read; unread matmuls are not eliminated.
            for wi in range(24):
                wps = psmm.tile([128, 512], F32, tag="mm", name=f"warm{wi}")
                nc.tensor.matmul(
                    wps, lhsT=ident, rhs=tri[:, 0, :], start=True, stop=True
                )

            # persistent projection outputs (kT/mv1 span all tokens;
            # qT only lives for its own 512-token q-chunk group)
            kT_all = big.tile([128, 4, S], F32R)
            mv1 = big.tile([128, NTT, HPC, DV + 1], F32R)  # [k128, ktile, h, dv+1]

            # weights loaded lazily at first use (interleaved per-dc with the
            # first x chunk so the first matmul's operands arrive first)
            w_sb = {}

            def project_chunk(tcn, qT_j):
                """Project token chunk [tcn*TC, (tcn+1)*TC) for q, k, v."""
                for name in "vqk":
                    first_w = name not in w_sb
                    if first_w:
                        w_sb[name] = wtpool.tile(
                            [128, NDC, FPC], F32R, tag=f"w{name}", name=f"w{name}"
                        )
                    halves = [
                        xtpool.tile([128, 4, TC], F32R, tag="x", name=f"x{name}{tcn}l"),
                        xtpool.tile([128, 4, TC], F32R, tag="x", name=f"x{name}{tcn}h"),
                    ]
                    x_at = lambda dc: halves[dc // 4][:, dc % 4, :]
                    if first_w:
                        # interleave weight and x slices per-dc so the first
                        # matmul's operands arrive before the bulk of the data
                        for dc in range(NDC):
                            nc.sync.dma_start(
                                out=w_sb[name][:, dc, :],
                                in_=bass.AP(
                                    tensor=wt[name],
                                    offset=dc * 128 * FPC,
                                    ap=[[FPC, 128], [1, FPC]],
                                ),
                            )
                            nc.sync.dma_start(
                                out=x_at(dc),
                                in_=bass.AP(
                                    tensor=xt[name],
                                    offset=tcn * TC + dc * 128 * S,
                                    ap=[[S, 128], [1, TC]],
                                ),
                            )
                    else:
                        for hv in range(2):
                            nc.sync.dma_start(
                                out=halves[hv],
                                in_=bass.AP(
                                    tensor=xt[name],
                                    offset=tcn * TC + hv * 4 * 128 * S,
                                    ap=[[S, 128], [128 * S, 4], [1, TC]],
                                ),
                            )
                    if name in "qk":
                        for fc in range(4):
                            ps = psmm.tile([128, TC], F32, tag="mm")
                            for dc in range(NDC):
                                nc.tensor.matmul(
                                    ps,
                                    lhsT=w_sb[name][:, dc, fc * 128 : (fc + 1) * 128],
                                    rhs=x_at(dc),
                                    start=(dc == 0),
                                    stop=(dc == NDC - 1),
                                )
                            ts = slice(0, TC)
                            if name == "q":
                                # split into per-head zero-padded blocks (DVE:
                                # keeps ACT free for exp)
                                for hh in range(2):
                                    hp = hh * 64
                                    o = qT_j[hp : hp + 64, 2 * fc + hh, ts]
                                    i_ = ps[hp : hp + 64, :]
                                    if name in bias_sb:
                                        nc.vector.tensor_scalar_add(
                                            o, i_, bias_sb[name][hp : hp + 64, fc : fc + 1]
                                        )
                                    else:
                                        nc.vector.tensor_copy(o, i_)
                            else:
                                o = kT_all[:, fc, tcn * TC : (tcn + 1) * TC]
                                if name in bias_sb:
                                    nc.scalar.activation(
                                        o, ps, IDENT_FN, bias=bias_sb[name][:, fc : fc + 1]
                                    )
                                else:
                                    nc.scalar.activation(o, ps, COPY)
                    else:
                        for tt in range(TC // 128):
                            t = tcn * (TC // 128) + tt
                            ps = psmm.tile([128, FPC], F32, tag="mm")
                            for dc in range(NDC):
                                nc.tensor.matmul(
                                    ps,
                                    lhsT=x_at(dc)[:, tt * 128 : (tt + 1) * 128],
                                    rhs=w_sb[name][:, dc, :],
                                    start=(dc == 0),
                                    stop=(dc == NDC - 1),
                                )
                            if bv_d is not None:
                                nc.vector.tensor_add(ps, ps, bv_b)
                            for h in range(HPC):
                                nc.vector.tensor_scalar_mul(
                                    mv1[:, t, h, 0:DV],
                                    ps[:, h * DV : (h + 1) * DV],
                                    maskcol[:, t : t + 1],
                                )
                                nc.gpsimd.tensor_copy(
                                    mv1[:, t, h, DV : DV + 1], maskcol[:, t : t + 1]
                                )

            def attention_pair(fc, j, qT_j, finalize, outsb):
                h0, h1 = 2 * fc, 2 * fc + 1
                nkt = 4 * (j + 1)
                ctxs = [
                    psctx.tile([DV + 1, 512], F32, tag="ctx", name=f"ctx{fc}_{j}_0"),
                    psctx.tile([DV + 1, 512], F32, tag="ctx", name=f"ctx{fc}_{j}_1"),
                ]
                for kt in range(nkt):
                    p = kt - 4 * j
                    off = DIAG_OFF[p] if p >= 0 else 0
                    qs = slice(off, 512)
                    scs = []
                    for hh, h in enumerate((h0, h1)):
                        sc = psmm.tile([128, 512], F32, tag="mm", name=f"sc{fc}_{j}_{kt}_{hh}")
                        nc.tensor.matmul(
                            sc[:, off:],
                            lhsT=kT_all[:, fc, kt * 128 : (kt + 1) * 128],
                            rhs=qT_j[:, h, qs],
                            start=True,
                            stop=True,
                        )
                        scs.append(sc)
                    Es = []
                    for hh, h in enumerate((h0, h1)):
                        E = epool.tile([128, 512], F32R, tag="e", name=f"e{fc}_{j}_{kt}_{hh}")
                        nc.scalar.activation(E[:, off:], scs[hh][:, off:], EXP, scale=float(SCALE))
                        if p >= 0:
                            nc.vector.tensor_mul(E[:, off:], E[:, off:], tri[:, p, off:])
                        Es.append(E)
                    for hh, h in enumerate((h0, h1)):
                        nc.tensor.matmul(
                            ctxs[hh][:, off:],
                            lhsT=mv1[:, kt, h, :],
                            rhs=Es[hh][:, off:],
                            start=(kt == 0),
                            stop=(kt == nkt - 1),
                        )
                for hh, h in enumerate((h0, h1)):
                    finalize(h, j, ctxs[hh], outsb)

            def finalize(h, j, ctx, outsb):
                # ---- finalize this (head, q-chunk)
                ctx_sb = fin.tile([DV + 1, 512], F32R, tag="ctxsb")
                nc.vector.tensor_copy(ctx_sb, ctx)
                sums4 = fin.tile([4, 128], F32R, tag="sums", bufs=2)
                nc.sync.dma_start(
                    out=sums4,
                    in_=ctx_sb[DV : DV + 1, :].rearrange("p (a b) -> p a b", a=4),
                )
                rec = fin.tile([4, 128], F32R, tag="rec", bufs=2)
                nc.vector.tensor_scalar_add(rec, sums4, 1e-8)
                with nc.allow_low_precision(reason="f32r is bit-identical fp32"):
                    nc.vector.reciprocal(rec, rec)
                recT_ps = pstp.tile([128, 4], F32R, tag="tp", name=f"recT_ps{h}_{j}")
                nc.tensor.transpose(recT_ps, rec, ident[0:4, 0:4])
                recT = fin.tile([128, 4], F32, tag="recT")
                nc.vector.tensor_copy(recT, recT_ps)
                for tt in range(4):
                    ctxn = pstp.tile([128, DV], F32R, tag="tp", name=f"ctxn{h}_{j}_{tt}")
                    nc.tensor.transpose(
                        ctxn,
                        ctx_sb[0:DV, tt * 128 : (tt + 1) * 128],
                        ident[0:DV, 0:DV],
                    )
                    nc.vector.tensor_scalar_mul(
                        outsb[tt][:, h * DV : (h + 1) * DV], ctxn, recT[:, tt : tt + 1]
                    )

            # software pipeline: project 512 tokens, then attend at that
            # q-chunk for all heads (next group's projections fill PE gaps)
            # per-head q^T blocks, zero-padded to K=128: the unused 64
            # partition rows must be zero so a full-K matmul (which keeps
            # the PE clock-gate warm; K=64 streams at half rate) kills the
            # other head's kT rows
            def new_qtile(j):
                qT_j = qtpool.tile([128, HPC, 512], F32R, tag="qt", name=f"qt{j}")
                for h in range(HPC):
                    zp = 64 - (h % 2) * 64
                    nc.vector.tensor_copy(qT_j[zp : zp + 64, h, :], zeros_sb)
                return qT_j

            # emission order interleaves next group's projection chunks
            # between this group's attention pairs so the scheduler has
            # PE filler work at matching priority during the (ACT-paced)
            # attention stretches
            qtiles = {0: new_qtile(0)}
            project_chunk(0, qtiles[0])
            for j in range(NQC):
                qT_j = qtiles[j]
                outsb = [
                    fin.tile([128, FPC], F32R, tag="outsb", name=f"outsb{j}_{tt}", bufs=6)
                    for tt in range(4)
                ]
                for fc in range(4):
                    attention_pair(fc, j, qT_j, finalize, outsb)
                    if j + 1 < NQC and fc == 0:
                        qtiles[j + 1] = new_qtile(j + 1)
                        project_chunk(j + 1, qtiles[j + 1])
                for tt in range(4):
                    nc.sync.dma_start(
                        out=out_d[(j * 4 + tt) * 128 : (j * 4 + tt + 1) * 128, :],
                        in_=outsb[tt],
                    )
    nc.finalize()
    return nc


_NC_CACHE = {}


def _get_nc(flags):
    if flags not in _NC_CACHE:
        _NC_CACHE[flags] = build_nc(*flags)
    return _NC_CACHE[flags]


def _host_consts():
    tri = np.zeros((4, 128, 512), np.float32)
    for p in range(4):
        for kk in range(128):
            tri[p, kk, kk + 128 * p :] = 1.0
    return tri, np.eye(128, dtype=np.float32)


def kernel(Q, K, V, mask, W_Q, W_K, W_V, b_Q, b_K, b_V, _run=None):
    Q, K, V = (np.asarray(a, np.float32) for a in (Q, K, V))
    W_Q, W_K, W_V = (np.asarray(a, np.float32) for a in (W_Q, W_K, W_V))
    b_Q, b_K, b_V = (np.asarray(a, np.float32) for a in (b_Q, b_K, b_V))
    mask = np.asarray(mask)

    flags = (bool(b_Q.any()), bool(b_K.any()), bool(b_V.any()))
    nc = _get_nc(flags)
    tri, ident = _host_consts()

    in_maps = []
    for c in range(8):
        b, half = c // 2, c % 2
        fsl = slice(half * FPC, (half + 1) * FPC)
        m = {
            "xtq": np.ascontiguousarray(Q[b].T),
            "xtk": np.ascontiguousarray(K[b].T),
            "xtv": np.ascontiguousarray(V[b].T),
            "wtq": np.ascontiguousarray(W_Q[fsl].T),
            "wtk": np.ascontiguousarray(W_K[fsl].T),
            "wtv": np.ascontiguousarray(W_V[fsl].T),
            "maskf": mask[b].astype(np.float32),
            "tri": tri,
            "ident": ident,
        }
        if flags[0]:
            m["bq"] = b_Q[fsl]
        if flags[1]:
            m["bk"] = b_K[fsl]
        if flags[2]:
            m["bv"] = b_V[:DV]
        in_maps.append(m)

    # the kernel broadcasts one [DV] b_V vector across heads; exact only when
    # b_V is constant across heads (it is zeros in this problem)
    if flags[2]:
        bv_heads = b_V.reshape(H, DV)
        assert np.allclose(bv_heads, bv_heads[0]), "per-head b_V unsupported"

    run = _run or (lambda n, im: run_bass_kernel_spmd(n, im, core_ids=list(range(8))))
    res = run(nc, in_maps)

    out = np.empty((B, S, H * DV), np.float32)
    for c in range(8):
        b, half = c // 2, c % 2
        out[b, :, half * FPC : (half + 1) * FPC] = res.results[c]["out"]
    return out

